# revision 9
# baseline (speedup 1.0000x reference)
"""CharRNN (GRU, reset_after=True) Trainium2 kernel.

Sharding: pure data parallel over batch (4096 -> 8 cores x 512).

Host precomputes xW = ktab[x] (ktab = kernel + input_bias, plus the z/r
recurrent biases folded in), since one_hot(x) @ kernel is exactly a row
gather. The device runs only the serial GRU recurrence.

Device layout is feature-major: [h-feature partitions, batch free-dim].
Per step (all tiles on partitions 0:20, base-0 as the HW requires):

  psA = U_z^T h (+ ones-row bias)  then  += I^T xw_z   (matmul inject)
  psB = U_r^T h                    then  += I^T xw_r
  psC = U_h^T h + br_h (ones-row)
  z = sigmoid(psA); r = sigmoid(psB)          (ACT, reads PSUM)
  t1 = r * psC; t2 = t1 + xw_h                (DVE)
  hc = tanh(t2)                               (ACT)
  h' = hc + z * (h - hc)                      (DVE x3, in-place h)

The xw injects use identity-weight matmuls so the sigmoid args accumulate
entirely in PSUM (saves two DVE adds per step).
"""

import numpy as np

import concourse.bacc as bacc
import concourse.tile as tile
from concourse import mybir
from concourse.bass_utils import run_bass_kernel_spmd

B, T, V, H, L = 4096, 256, 256, 20, 15
NCORES = 8
BC = B // NCORES          # 512 batch per core
H3 = 3 * H
TC = 16                   # time steps per DMA chunk
NCHUNK = T // TC

_CACHE = {}


def _build_program():
    nc = bacc.Bacc("TRN2", target_bir_lowering=False, debug=False)
    f32 = mybir.dt.float32
    AF = mybir.ActivationFunctionType

    # xwzr: [chunk, 2H, TC, BC] rows 0:20 = xw_z, 20:40 = xw_r (gate biases folded)
    xwzr = nc.dram_tensor("xwzr", [NCHUNK, 2 * H, TC, BC], f32, kind="ExternalInput")
    xwh = nc.dram_tensor("xwh", [NCHUNK, H, TC, BC], f32, kind="ExternalInput")
    # wz/wr/wh: [H+1, H] rows 0:20 = U_gate, row 20 = recurrent bias (h gate only)
    wz = nc.dram_tensor("wz", [H + 1, H], f32, kind="ExternalInput")
    wr = nc.dram_tensor("wr", [H + 1, H], f32, kind="ExternalInput")
    wh = nc.dram_tensor("wh", [H + 1, H], f32, kind="ExternalInput")
    eye = nc.dram_tensor("eye", [2 * H, 2 * H], f32, kind="ExternalInput")
    ones = nc.dram_tensor("ones", [1, BC], f32, kind="ExternalInput")
    dw = nc.dram_tensor("dw", [H, L], f32, kind="ExternalInput")
    db = nc.dram_tensor("db", [L, 1], f32, kind="ExternalInput")
    out = nc.dram_tensor("out", [L, BC], f32, kind="ExternalOutput")

    with tile.TileContext(nc) as tc:
        with (
            tc.tile_pool(name="consts", bufs=1) as consts,
            tc.tile_pool(name="xw", bufs=2) as xwpool,
            tc.tile_pool(name="work", bufs=3) as work,
            tc.tile_pool(name="psum", bufs=2, space="PSUM") as psum,
            tc.tile_pool(name="psum1", bufs=1, space="PSUM") as psum1,
        ):
            wz_sb = consts.tile([H + 1, H], f32)
            wr_sb = consts.tile([H + 1, H], f32)
            wh_sb = consts.tile([H + 1, H], f32)
            eye_sb = consts.tile([2 * H, 2 * H], f32)
            dw_sb = consts.tile([H, L], f32)
            db_sb = consts.tile([L, 1], f32)
            nc.sync.dma_start(out=wz_sb, in_=wz.ap())
            nc.sync.dma_start(out=wr_sb, in_=wr.ap())
            nc.sync.dma_start(out=wh_sb, in_=wh.ap())
            nc.sync.dma_start(out=eye_sb, in_=eye.ap())
            nc.sync.dma_start(out=dw_sb, in_=dw.ap())
            nc.sync.dma_start(out=db_sb, in_=db.ap())

            # Persistent state [H+1, BC]: rows 0:20 = h, row 20 = const 1.0
            # (folds per-gate ones-row biases into the matmuls). Row 20 is
            # written by DMA because compute engines cannot address base-20.
            h_sb = consts.tile([H + 1, BC], f32)
            nc.vector.memset(h_sb[0:H, :], 0.0)
            nc.sync.dma_start(out=h_sb[H : H + 1, :], in_=ones.ap())

            for ci in range(NCHUNK):
                xwzr_sb = xwpool.tile([2 * H, TC, BC], f32, tag="xwzr")
                xwh_sb = xwpool.tile([H, TC, BC], f32, tag="xwh")
                nc.sync.dma_start(out=xwzr_sb, in_=xwzr.ap()[ci])
                nc.sync.dma_start(out=xwh_sb, in_=xwh.ap()[ci])
                for tt in range(TC):
                    ps_a = psum.tile([H, BC], f32, tag="ps_a")
                    ps_b = psum.tile([H, BC], f32, tag="ps_b")
                    ps_c = psum.tile([H, BC], f32, tag="ps_c")
                    # xw injects (independent of h -> off critical path)
                    nc.tensor.matmul(
                        ps_a, eye_sb[:, 0:H], xwzr_sb[:, tt, :],
                        start=True, stop=False,
                    )
                    nc.tensor.matmul(
                        ps_b, eye_sb[:, H : 2 * H], xwzr_sb[:, tt, :],
                        start=True, stop=False,
                    )
                    # recurrent matmuls (accumulate onto the injected xw)
                    nc.tensor.matmul(ps_a, wz_sb, h_sb, start=False, stop=True)
                    nc.tensor.matmul(ps_b, wr_sb, h_sb, start=False, stop=True)
                    nc.tensor.matmul(ps_c, wh_sb, h_sb, start=True, stop=True)

                    z = work.tile([H, BC], f32, tag="z")
                    r = work.tile([H, BC], f32, tag="r")
                    nc.scalar.activation(z, ps_a, AF.Sigmoid)
                    nc.scalar.activation(r, ps_b, AF.Sigmoid)

                    t1 = work.tile([H, BC], f32, tag="t1")
                    nc.vector.tensor_mul(t1, r, ps_c)
                    t2 = work.tile([H, BC], f32, tag="t2")
                    nc.vector.tensor_add(t2, t1, xwh_sb[:, tt, :])
                    hc = work.tile([H, BC], f32, tag="hc")
                    nc.scalar.activation(hc, t2, AF.Tanh)

                    d = work.tile([H, BC], f32, tag="d")
                    nc.vector.tensor_sub(d, h_sb[0:H, :], hc)
                    m = work.tile([H, BC], f32, tag="m")
                    nc.vector.tensor_mul(m, z, d)
                    nc.vector.tensor_add(h_sb[0:H, :], hc, m)

            ps_out = psum1.tile([L, BC], f32, tag="ps_out")
            nc.tensor.matmul(ps_out, dw_sb, h_sb[0:H, :], start=True, stop=True)
            out_sb = work.tile([L, BC], f32, tag="out_sb")
            nc.scalar.activation(out_sb, ps_out, AF.Identity, bias=db_sb[:, 0:1])
            nc.sync.dma_start(out=out.ap(), in_=out_sb)

    nc.compile()
    return nc


def _get_program():
    if "nc" not in _CACHE:
        _CACHE["nc"] = _build_program()
    return _CACHE["nc"]


def _prepare_inputs(x, kernel, recurrent_kernel, bias, dense_w, dense_b):
    x = np.asarray(x)
    kernel = np.asarray(kernel, dtype=np.float32)
    recurrent_kernel = np.asarray(recurrent_kernel, dtype=np.float32)
    bias = np.asarray(bias, dtype=np.float32)
    dense_w = np.asarray(dense_w, dtype=np.float32)
    dense_b = np.asarray(dense_b, dtype=np.float32)

    # Fold input bias (all gates) + recurrent bias (z/r only) into the table.
    ktab = kernel + bias[0]
    ktab[:, 0 : 2 * H] += bias[1][0 : 2 * H]

    def aug(u, brow):
        return np.ascontiguousarray(
            np.concatenate([u, brow[None, :]], axis=0).astype(np.float32)
        )

    zrow = np.zeros(H, np.float32)
    wz_np = aug(recurrent_kernel[:, 0:H], zrow)
    wr_np = aug(recurrent_kernel[:, H : 2 * H], zrow)
    wh_np = aug(recurrent_kernel[:, 2 * H : H3], bias[1][2 * H : H3])
    eye_np = np.eye(2 * H, dtype=np.float32)
    ones_np = np.ones((1, BC), np.float32)
    dw_np = np.ascontiguousarray(dense_w)
    db_np = np.ascontiguousarray(dense_b[:, None])

    in_maps = []
    for c in range(NCORES):
        xc = x[c * BC : (c + 1) * BC]          # [BC, T]
        xw = ktab[xc]                          # [BC, T, H3]
        # [BC, T, H3] -> [T, H3, BC] -> [NCHUNK, H3, TC, BC]
        xw = xw.transpose(1, 2, 0).reshape(NCHUNK, TC, H3, BC).transpose(0, 2, 1, 3)
        in_maps.append(
            {
                "xwzr": np.ascontiguousarray(xw[:, 0 : 2 * H]),
                "xwh": np.ascontiguousarray(xw[:, 2 * H : H3]),
                "wz": wz_np,
                "wr": wr_np,
                "wh": wh_np,
                "eye": eye_np,
                "ones": ones_np,
                "dw": dw_np,
                "db": db_np,
            }
        )
    return in_maps


def run(inputs, trace=False):
    nc = _get_program()
    in_maps = _prepare_inputs(
        inputs["x"],
        inputs["kernel"],
        inputs["recurrent_kernel"],
        inputs["bias"],
        inputs["dense_w"],
        inputs["dense_b"],
    )
    res = run_bass_kernel_spmd(nc, in_maps, core_ids=list(range(NCORES)), trace=trace)
    logits = np.empty((B, L), dtype=np.float32)
    for c in range(NCORES):
        logits[c * BC : (c + 1) * BC] = res.results[c]["out"].T
    return logits, res.exec_time_ns


def kernel(**inputs) -> np.ndarray:
    logits, _ = run(inputs, trace=False)
    return logits


# revision 13
# speedup vs baseline: 2.5834x; 2.5834x over previous
"""CharRNN (GRU, reset_after=True) Trainium2 kernel.

Sharding: pure data parallel over batch (4096 -> 8 cores x 512).

Host precomputes xW = ktab[x] (ktab = kernel + input_bias + z/r recurrent
bias), since one_hot(x) @ kernel is exactly a row gather. The device runs
only the serial GRU recurrence, in fp16 (fp32 PSUM accumulation).

Device layout is feature-major: [h-feature partitions, batch free].
The per-chunk rhs tensor packs, along partitions:
  rows 0:20   h-slots (slot t = h state entering step t; written on device)
  row  20     ones (folds br_h into the U_h matmul)
  rows 21:41  xw_r slices   rows 41:61  xw_z slices
One matmul (K=61) then produces BOTH sigmoid args (r at partitions 0:20,
z at 32:52) with xw already accumulated; a second small matmul produces
hh+br_h; a third relanes z from partitions 32:52 down to 0:20 (off the
critical path) so the final gate products are all lane-aligned.

Per step (per batch-group g, G groups pipeline the serial chain):
  psAB = W1^T rhs_aug[:,t]   (r-args | z-args)
  psC  = W2^T rhs_aug[0:33,t] (hh + br_h)
  zr   = sigmoid(psAB[0:52])            r = zr[0:20]
  psZ  = Zrel^T zr                      (z relaned to 0:20)
  t1 = r * psC ; t2 = t1 + xw_h ; hc = tanh(t2)
  d = h - hc ; m = psZ * d ; h' = hc + m -> next h-slot
"""

import numpy as np
import ml_dtypes

import concourse.bacc as bacc
import concourse.tile as tile
from concourse import mybir
from concourse.bass_utils import run_bass_kernel_spmd

B, T, V, H, L = 4096, 256, 256, 20, 15
NCORES = 8
BC = B // NCORES          # 512 batch per core
H3 = 3 * H
TC = 16                   # time steps per DMA chunk
NCHUNK = T // TC
G = 2                     # batch groups pipelined per core
BG = BC // G

KA = 61                   # augmented K: 20 h + 1 ones + 20 xw_r + 20 xw_z

_CACHE = {}


def _build_program():
    nc = bacc.Bacc("TRN2", target_bir_lowering=False, debug=False)
    f16 = mybir.dt.float16
    f32 = mybir.dt.float32
    AF = mybir.ActivationFunctionType

    # host block: [chunk, 41, TC, BG] per group -> SBUF rows 32:73
    xa = [
        nc.dram_tensor(f"xa{g}", [NCHUNK, 41, TC, BG], f16, kind="ExternalInput")
        for g in range(G)
    ]
    xh = [
        nc.dram_tensor(f"xh{g}", [NCHUNK, H, TC, BG], f16, kind="ExternalInput")
        for g in range(G)
    ]
    w1 = nc.dram_tensor("w1", [KA, 52], f16, kind="ExternalInput")
    w2 = nc.dram_tensor("w2", [21, H], f16, kind="ExternalInput")
    zrel = nc.dram_tensor("zrel", [52, H], f16, kind="ExternalInput")
    dw = nc.dram_tensor("dw", [H, L], f16, kind="ExternalInput")
    db = nc.dram_tensor("db", [L, 1], f32, kind="ExternalInput")
    out = nc.dram_tensor("out", [L, BC], f32, kind="ExternalOutput")

    with tile.TileContext(nc) as tc:
        with (
            tc.tile_pool(name="consts", bufs=1) as consts,
            tc.tile_pool(name="rhs", bufs=2) as rhspool,
            tc.tile_pool(name="work", bufs=3) as work,
            tc.tile_pool(name="psum", bufs=1, space="PSUM") as psum,
            tc.tile_pool(name="psum1", bufs=1, space="PSUM") as psum1,
        ):
            w1_sb = consts.tile([KA, 52], f16)
            w2_sb = consts.tile([21, H], f16)
            zrel_sb = consts.tile([52, H], f16)
            dw_sb = consts.tile([H, L], f16)
            db_sb = consts.tile([L, 1], f32)
            nc.sync.dma_start(out=w1_sb, in_=w1.ap())
            nc.sync.dma_start(out=w2_sb, in_=w2.ap())
            nc.sync.dma_start(out=zrel_sb, in_=zrel.ap())
            nc.sync.dma_start(out=dw_sb, in_=dw.ap())
            nc.sync.dma_start(out=db_sb, in_=db.ap())

            hfin = consts.tile([H, BC], f16)

            # per-group chunk tiles; cur/nxt for cross-chunk h handoff
            def alloc_chunk(ci):
                ts = []
                for g in range(G):
                    rt = rhspool.tile([KA, TC, BG], f16, tag=f"rhs{g}")
                    xt = rhspool.tile([H, TC, BG], f16, tag=f"xh{g}")
                    nc.sync.dma_start(out=rt[20:KA, :, :], in_=xa[g].ap()[ci])
                    nc.sync.dma_start(out=xt, in_=xh[g].ap()[ci])
                    ts.append((rt, xt))
                return ts

            cur = alloc_chunk(0)
            for g in range(G):
                nc.vector.memset(cur[g][0][0:H, 0, :], 0.0)

            for ci in range(NCHUNK):
                nxt = alloc_chunk(ci + 1) if ci + 1 < NCHUNK else None
                for tt in range(TC):
                    for g in range(G):
                        rt, xt = cur[g]
                        ps_ab = psum.tile([52, BG], f32, tag=f"ps_ab{g}")
                        ps_c = psum.tile([H, BG], f32, tag=f"ps_c{g}")
                        ps_z = psum.tile([H, BG], f32, tag=f"ps_z{g}")
                        nc.tensor.matmul(
                            ps_ab, w1_sb, rt[:, tt, :], start=True, stop=True
                        )
                        nc.tensor.matmul(
                            ps_c, w2_sb, rt[0:21, tt, :], start=True, stop=True
                        )

                        zr = work.tile([52, BG], f16, tag=f"zr{g}")
                        nc.scalar.activation(zr, ps_ab, AF.Sigmoid)
                        nc.tensor.matmul(ps_z, zrel_sb, zr, start=True, stop=True)

                        t1 = work.tile([H, BG], f16, tag=f"t1{g}")
                        nc.vector.tensor_mul(t1, zr[0:H, :], ps_c)
                        t2 = work.tile([H, BG], f16, tag=f"t2{g}")
                        nc.vector.tensor_add(t2, t1, xt[:, tt, :])
                        hc = work.tile([H, BG], f16, tag=f"hc{g}")
                        nc.scalar.activation(hc, t2, AF.Tanh)

                        d = work.tile([H, BG], f16, tag=f"d{g}")
                        nc.vector.tensor_sub(d, rt[0:H, tt, :], hc)
                        m = work.tile([H, BG], f16, tag=f"m{g}")
                        nc.vector.tensor_mul(m, ps_z, d)
                        if tt + 1 < TC:
                            hdst = rt[0:H, tt + 1, :]
                        elif nxt is not None:
                            hdst = nxt[g][0][0:H, 0, :]
                        else:
                            hdst = hfin[:, g * BG : (g + 1) * BG]
                        nc.vector.tensor_add(hdst, hc, m)
                cur = nxt

            ps_out = psum1.tile([L, BC], f32, tag="ps_out")
            nc.tensor.matmul(ps_out, dw_sb, hfin, start=True, stop=True)
            out_sb = work.tile([L, BC], f32, tag="out_sb")
            nc.scalar.activation(out_sb, ps_out, AF.Identity, bias=db_sb[:, 0:1])
            nc.sync.dma_start(out=out.ap(), in_=out_sb)

    nc.compile()
    return nc


def _get_program():
    if "nc" not in _CACHE:
        _CACHE["nc"] = _build_program()
    return _CACHE["nc"]


def _prepare_inputs(x, kernel, recurrent_kernel, bias, dense_w, dense_b):
    x = np.asarray(x)
    kernel = np.asarray(kernel, dtype=np.float32)
    rk = np.asarray(recurrent_kernel, dtype=np.float32)
    bias = np.asarray(bias, dtype=np.float32)
    f16 = np.float16

    # table with input bias (all gates) + recurrent bias (z/r) folded in
    ktab = kernel + bias[0]
    ktab[:, 0 : 2 * H] += bias[1][0 : 2 * H]
    ktab = ktab.astype(f16)

    uz = rk[:, 0:H]
    ur = rk[:, H : 2 * H]
    uh = rk[:, 2 * H : H3]

    # W1 [61, 52]: rows 0:20 h -> (Ur | cols 32:52 Uz); rows 21:41 xw_r -> I;
    # rows 41:61 xw_z -> I at cols 32:52. Row 20 (ones) unused here.
    w1_np = np.zeros((KA, 52), np.float32)
    w1_np[0:H, 0:H] = ur
    w1_np[0:H, 32:52] = uz
    w1_np[21:41, 0:H] = np.eye(H)
    w1_np[41:61, 32:52] = np.eye(H)
    # W2 [21, 20]: rows 0:20 -> Uh; row 20 -> br_h
    w2_np = np.zeros((21, H), np.float32)
    w2_np[0:H, :] = uh
    w2_np[20, :] = bias[1][2 * H : H3]
    # zrel [52, 20]: rows 32:52 -> I
    zrel_np = np.zeros((52, H), np.float32)
    zrel_np[32:52, 0:H] = np.eye(H)

    common = {
        "w1": w1_np.astype(f16),
        "w2": w2_np.astype(f16),
        "zrel": zrel_np.astype(f16),
        "dw": np.ascontiguousarray(np.asarray(dense_w, np.float32)).astype(f16),
        "db": np.ascontiguousarray(np.asarray(dense_b, np.float32)[:, None]),
    }

    in_maps = []
    for c in range(NCORES):
        xc = x[c * BC : (c + 1) * BC]          # [BC, T]
        xw = ktab[xc]                          # [BC, T, 60] f16
        # -> [T, 60, BC] -> [NCHUNK, TC, 60, BC]
        xw = xw.transpose(1, 2, 0).reshape(NCHUNK, TC, H3, BC)
        m = dict(common)
        for g in range(G):
            sl = xw[:, :, :, g * BG : (g + 1) * BG]   # [NCHUNK, TC, 60, BG]
            blk = np.zeros((NCHUNK, 41, TC, BG), f16)
            blk[:, 0, :, :] = 1.0                      # ones row (row 20)
            blk[:, 1 : 1 + H] = sl[:, :, H : 2 * H].transpose(0, 2, 1, 3)   # xw_r
            blk[:, 1 + H : 41] = sl[:, :, 0:H].transpose(0, 2, 1, 3)        # xw_z
            m[f"xa{g}"] = np.ascontiguousarray(blk)
            m[f"xh{g}"] = np.ascontiguousarray(
                sl[:, :, 2 * H : H3].transpose(0, 2, 1, 3)
            )
        in_maps.append(m)
    return in_maps


def run(inputs, trace=False):
    nc = _get_program()
    in_maps = _prepare_inputs(
        inputs["x"],
        inputs["kernel"],
        inputs["recurrent_kernel"],
        inputs["bias"],
        inputs["dense_w"],
        inputs["dense_b"],
    )
    res = run_bass_kernel_spmd(nc, in_maps, core_ids=list(range(NCORES)), trace=trace)
    logits = np.empty((B, L), dtype=np.float32)
    for c in range(NCORES):
        logits[c * BC : (c + 1) * BC] = res.results[c]["out"].T
    return logits, res.exec_time_ns


def kernel(**inputs) -> np.ndarray:
    logits, _ = run(inputs, trace=False)
    return logits
